# revision 1
# baseline (speedup 1.0000x reference)
"""Trainium2 Bass kernel for nn_BiGruBNattMaxFocalNet.

Data-parallel over batch: B=32 -> 4 per core x 8 cores.
Per-core pipeline (all feature-on-partition "family P" layouts):
  1. encoder input projections (f32r matmuls)
  2. shared BiGRU scans over context (T=100, L=4) and options (T=50, L=20)
  3. ctx_key / query projections
  4. per-(b,k) Bahdanau energy: DVE outer-add + ACT tanh + PE w_e-reduce
  5. exp / row-col sums / normalized attention matmuls
  6. fc/fo features -> attention-GRU input projections
  7. attention BiGRU scans with running max
  8. logits + softmax over K
"""
import numpy as np
from contextlib import ExitStack

import concourse.bass as bass
import concourse.tile as tile
from concourse import mybir, masks
from concourse.bass_utils import run_bass_kernel_spmd
from concourse.vector_clock import ScopedClock

f32 = mybir.dt.float32
f32r = mybir.dt.float32r
bf16 = mybir.dt.bfloat16
AF = mybir.ActivationFunctionType
ALU = mybir.AluOpType

H, H2, H3, E, LC, LO, KOPT = 128, 256, 384, 300, 100, 50, 5
NCORES = 8
B4 = 4            # batch per core
LCTX = B4         # ctx scan lanes
LOPT = B4 * KOPT  # option scan lanes (=20)
NCTX = LC * LCTX      # 400 ctx (t,b) cols
NOPT = LO * LOPT      # 1000 opt (t,l) cols
NFC = LC * LOPT       # 2000 fc cols
NFO = LO * LOPT       # 1000 fo cols

# dtype used for the tanh-energy tiles consumed by the w_e-reduce matmul
# (f32r keeps the reduce at 1 cycle/row with ~1e-4 rounding)
S_DT = f32r


class TC(tile.TileContext):
    """TileContext with walrus-compatible tail drain (<=1 wait per inst)."""

    def _drain_and_barrier(self, tick_clock, wait_clock):
        nc = self.nc
        probe = nc.sync.nop(nofuse=True)
        wait_clock.add_sem_waits(
            probe.ins, ScopedClock({None: tick_clock.global_clock})
        )
        si = probe.ins.sync_info
        waits = list(si.on_wait or [])
        si.on_wait = []
        assert self.sems is not None
        by_name = {h.name: h for h in self.sems.allocated().values()}
        for w in waits:
            nc.sync.wait_ge(by_name[w.ant_name], w.wait_value)
        nc.sync.drain()
        nc.all_engine_barrier()
        popped = nc._tile_sem_poison_stack.pop()
        assert popped is self._sem_poison
        nc.clear_and_free_semaphores(list(self.sems.allocated().values()))
        nc.all_engine_barrier()


def split_multi_waits(nc, max_waits=1):
    """This walrus build rejects >1 sync-wait per instruction; hoist extras
    onto same-engine NOPs placed immediately before the offender."""
    cnt = 0
    for fn in nc.m.functions:
        for bb in fn.blocks:
            insts = list(bb.instructions)
            out = []
            changed = False
            for inst in insts:
                si = inst.sync_info
                waits = list(si.on_wait) if si is not None and si.on_wait else []
                if len(waits) > max_waits:
                    changed = True
                    for w in waits[:-max_waits]:
                        cnt += 1
                        nop = mybir.InstNoOp(name=f"wait-split-{cnt}")
                        nop.engine = inst.engine
                        nop.sync_info = mybir.SyncInfo(on_wait=[w], on_update=[])
                        out.append(nop)
                    inst.sync_info = mybir.SyncInfo(
                        on_wait=waits[-max_waits:],
                        on_update=list(si.on_update or []),
                    )
                out.append(inst)
            if changed:
                bb.instructions = out
    return cnt


def _enc_projection(nc, ppj, xt_tiles, wihT, bias3, gx, segs):
    """gx[:, g, seg] = (x @ Wih.T + bias)[g-chunk] for each N-segment.

    xt_tiles: list of (tile, rows) K-chunks of x.T (f32r). wihT: list of
    K-chunk tiles [rows, 384] (f32r). bias3: [128, 3]. gx: [128, 3, N].
    """
    for (lo, n) in segs:
        for g in range(3):
            ps = ppj.tile([128, 512], f32, tag="proj", name="ps_proj")
            for kc, (xt, rows) in enumerate(xt_tiles):
                nc.tensor.matmul(
                    ps[:, 0:n],
                    wihT[kc][0:rows, g * 128:(g + 1) * 128],
                    xt[0:rows, lo:lo + n],
                    start=(kc == 0), stop=(kc == len(xt_tiles) - 1),
                )
            nc.vector.tensor_scalar(
                gx[:, g, lo:lo + n], ps[:, 0:n], bias3[:, g:g + 1], None, ALU.add
            )


def _gru_scan(nc, ctx, tc, T, L, gx_f, gx_b, whhT_f, whhT_b,
              bhhn_f, bhhn_b, outs=None, hmax=None, pfx=""):
    """Bidirectional GRU scan, family-P layout.

    gx_*: [128, 3, T*L] precomputed input projections (+rz biases, +bih_n).
    whhT_*: [128, 384] recurrent weights (fp32). bhhn_*: [128, 1] = b_hh[n].
    outs: None or (outs_f, outs_b) [128, T*L] tiles to store every h_t.
    hmax: None or (hmax_f, hmax_b) [128, L] running-max tiles (pre-init).
    """
    hp = ctx.enter_context(tc.tile_pool(name=f"h{pfx}", bufs=3))
    vp = ctx.enter_context(tc.tile_pool(name=f"v{pfx}", bufs=3))
    gp = ctx.enter_context(tc.tile_pool(name=f"g{pfx}", bufs=2, space="PSUM"))

    h0 = hp.tile([128, L], f32, tag="h0", name="h0")
    nc.vector.memset(h0[:], 0.0)
    h = {"f": h0, "b": h0}
    WT = {"f": whhT_f, "b": whhT_b}
    GX = {"f": gx_f, "b": gx_b}
    BN = {"f": bhhn_f, "b": bhhn_b}
    OUTS = {"f": None, "b": None} if outs is None else {"f": outs[0], "b": outs[1]}
    HM = {"f": None, "b": None} if hmax is None else {"f": hmax[0], "b": hmax[1]}

    for t in range(T):
        for d in ("f", "b"):
            col = (t if d == "f" else T - 1 - t) * L
            gates = gp.tile([128, 3, L], f32, tag=f"g{d}", name=f"gates{d}")
            for g in range(3):
                nc.tensor.matmul(
                    gates[:, g, :], WT[d][:, g * 128:(g + 1) * 128], h[d][:, 0:L],
                    start=True, stop=True,
                )
            srz = vp.tile([128, 2, L], f32, tag=f"srz{d}", name=f"srz{d}")
            nc.vector.tensor_add(srz[:], gates[:, 0:2, :], GX[d][:, 0:2, col:col + L])
            rz = vp.tile([128, 2, L], f32, tag=f"rz{d}", name=f"rz{d}")
            nc.scalar.activation(rz[:], srz[:], AF.Sigmoid)
            tn = vp.tile([128, L], f32, tag=f"tn{d}", name=f"tn{d}")
            nc.vector.scalar_tensor_tensor(
                tn[:], gates[:, 2, :], BN[d][:, 0:1], rz[:, 0, :], ALU.add, ALU.mult
            )
            sn = vp.tile([128, L], f32, tag=f"sn{d}", name=f"sn{d}")
            nc.vector.tensor_add(sn[:], tn[:], GX[d][:, 2, col:col + L])
            n_t = vp.tile([128, L], f32, tag=f"n{d}", name=f"n{d}")
            nc.scalar.activation(n_t[:], sn[:], AF.Tanh)
            dd = vp.tile([128, L], f32, tag=f"dd{d}", name=f"dd{d}")
            nc.vector.tensor_sub(dd[:], h[d][:, 0:L], n_t[:])
            ee = vp.tile([128, L], f32, tag=f"ee{d}", name=f"ee{d}")
            nc.vector.tensor_mul(ee[:], rz[:, 1, :], dd[:])
            if OUTS[d] is not None:
                hn = OUTS[d][:, col:col + L]
            else:
                hn = hp.tile([128, L], f32, tag=f"h{d}", name=f"h{d}")[:, 0:L]
            nc.vector.tensor_add(hn, n_t[:], ee[:])
            if HM[d] is not None:
                nc.vector.tensor_tensor(HM[d][:, 0:L], HM[d][:, 0:L], hn, ALU.max)
            h[d] = hn


DEBUG = False
_BUILT = {}


def _build():
    nc = bass.Bass("TRN2", target_bir_lowering=False, debug=False)
    dram = {}

    def din(name, shape):
        dram[name] = nc.dram_tensor(name, list(shape), f32, kind="ExternalInput").ap()
        return dram[name]

    # sharded activations (host pre-transposed)
    din("ctxT", [E, NCTX])      # (e, (t, b))
    din("optT", [E, NOPT])      # (e, (t, k*4+b))
    # encoder weights
    for d in ("f", "b"):
        din(f"wihT_{d}", [E, H3])
        din(f"whhT_{d}", [H, H3])
        din(f"bias3_{d}", [H, 3])     # [:,0:2]=bih+bhh rz, [:,2]=bih_n
        din(f"bhhn_{d}", [H, 1])
        din(f"awihT_{d}", [8 * H, H3])
        din(f"awhhT_{d}", [H, H3])
        din(f"abias3_{d}", [H, 3])
        din(f"abhhn_{d}", [H, 1])
    din("wkT", [H2, H2])
    din("wqT", [H2, H2])
    din("wemat", [H2, H2])
    din("vvec", [H2, 1])
    din("wsimT", [4 * H, 1])
    out_ap = nc.dram_tensor("out", [B4, KOPT], f32, kind="ExternalOutput").ap()
    dbg = {}
    if DEBUG:
        for nm, shape in [
            ("d_ctxf", [H, NCTX]), ("d_ctxb", [H, NCTX]),
            ("d_optf", [H, NOPT]), ("d_optb", [H, NOPT]),
            ("d_scores", [LO, LC]), ("d_E", [LO, LC]),
            ("d_ck", [H, 2, NCTX]), ("d_q", [H, 2, NOPT]), ("d_we", [H, 2]),
            ("d_S0", [H, 10, LC]),
            ("d_acx", [H, 2, NFC]), ("d_aop", [H, 2, NFO]),
            ("d_hcf", [H, LOPT]), ("d_hcb", [H, LOPT]),
            ("d_hof", [H, LOPT]), ("d_hob", [H, LOPT]),
            ("d_logits", [1, LOPT]),
        ]:
            dbg[nm] = nc.dram_tensor(nm, shape, f32, kind="ExternalOutput").ap()

    with TC(nc) as tc, ExitStack() as ctx:
        pw = ctx.enter_context(tc.tile_pool(name="pw", bufs=1))
        pm = ctx.enter_context(tc.tile_pool(name="pm", bufs=1))
        pj_ctx = ExitStack()
        ppj = pj_ctx.enter_context(tc.tile_pool(name="ppj", bufs=2, space="PSUM"))

        # ---- load weights ----
        W = {}
        for d in ("f", "b"):
            W[f"wihT_{d}"] = [pw.tile([128, H3], f32r, name=f"wih{d}{kc}")
                              for kc in range(3)]
            for kc in range(3):
                rows = min(128, E - kc * 128)
                nc.gpsimd.dma_start(W[f"wihT_{d}"][kc][0:rows, :],
                                    dram[f"wihT_{d}"][kc * 128:kc * 128 + rows, :])
            W[f"awihT_{d}"] = [pw.tile([128, H3], f32r, name=f"awih{d}{kc}")
                               for kc in range(8)]
            for kc in range(8):
                nc.gpsimd.dma_start(W[f"awihT_{d}"][kc][:],
                                    dram[f"awihT_{d}"][kc * 128:(kc + 1) * 128, :])
            for nm in (f"whhT_{d}", f"awhhT_{d}"):
                W[nm] = pw.tile([128, H3], f32, name=nm)
                nc.sync.dma_start(W[nm][:], dram[nm][:])
            for nm in (f"bias3_{d}", f"abias3_{d}"):
                W[nm] = pw.tile([128, 3], f32, name=nm)
                nc.sync.dma_start(W[nm][:], dram[nm][:])
            for nm in (f"bhhn_{d}", f"abhhn_{d}"):
                W[nm] = pw.tile([128, 1], f32, name=nm)
                nc.sync.dma_start(W[nm][:], dram[nm][:])
        for nm in ("wkT", "wqT"):
            W[nm] = [pw.tile([128, H2], f32r, name=f"{nm}{kc}") for kc in range(2)]
            for kc in range(2):
                nc.gpsimd.dma_start(W[nm][kc][:], dram[nm][kc * 128:(kc + 1) * 128, :])
        W["wsimT"] = [pw.tile([128, 1], f32r, name=f"wsimT{kc}") for kc in range(4)]
        for kc in range(4):
            nc.gpsimd.dma_start(W["wsimT"][kc][:], dram["wsimT"][kc * 128:(kc + 1) * 128, :])
        ident = pw.tile([128, 128], f32, name="ident")
        masks.make_identity(nc, ident[:])

        # w_e = We.T @ v, as two f32r [128,1] chunks (replicated to 128 cols
        # for the reduce-matmul lhsT).
        wemat = [pw.tile([128, H2], f32, name=f"wemat{kc}") for kc in range(2)]
        for kc in range(2):
            nc.sync.dma_start(wemat[kc][:], dram["wemat"][kc * 128:(kc + 1) * 128, :])
        vtile = pw.tile([128, 2], f32, name="vtile")
        nc.sync.dma_start(vtile[:], dram["vvec"][:].rearrange("(a p) o -> p (a o)", a=2))
        we_ps = ppj.tile([128, 512], f32, tag="proj", name="we_ps")
        for hc in range(2):
            for jc in range(2):
                nc.tensor.matmul(we_ps[:, hc:hc + 1],
                                 wemat[jc][:, hc * 128:(hc + 1) * 128],
                                 vtile[:, jc:jc + 1],
                                 start=(jc == 0), stop=(jc == 1))
        we = pw.tile([128, 2], f32, name="we")
        nc.vector.tensor_copy(we[:], we_ps[:, 0:2])
        weRep = []
        for hc in range(2):
            wr32 = pw.tile([128, 128], f32, name=f"wrep32_{hc}")
            nc.vector.tensor_copy(
                wr32[:], bass.AP(tensor=we.tensor, offset=we.offset + hc,
                                 ap=[list(we.ap[0]), [0, 128]]))
            wrr = pw.tile([128, 128], f32r, name=f"wrep_{hc}")
            nc.gpsimd.dma_start(wrr[:], wr32[:])
            weRep.append(wrr)

        # ---- load activations (f32r) ----
        penc_ctx = ExitStack()
        penc = penc_ctx.enter_context(tc.tile_pool(name="penc", bufs=1))
        ctxT = [penc.tile([128, NCTX], f32r, name=f"ctxT{kc}") for kc in range(3)]
        optT = [penc.tile([128, NOPT], f32r, name=f"optT{kc}") for kc in range(3)]
        for kc in range(3):
            rows = min(128, E - kc * 128)
            nc.gpsimd.dma_start(ctxT[kc][0:rows, :], dram["ctxT"][kc * 128:kc * 128 + rows, :])
            nc.gpsimd.dma_start(optT[kc][0:rows, :], dram["optT"][kc * 128:kc * 128 + rows, :])
        xt_ctx = [(ctxT[0], 128), (ctxT[1], 128), (ctxT[2], 44)]
        xt_opt = [(optT[0], 128), (optT[1], 128), (optT[2], 44)]

        # ---- encoder gx ----
        gx1c = {}
        gx1o = {}
        for d in ("f", "b"):
            gx1c[d] = penc.tile([128, 3, NCTX], f32, name=f"gx1c{d}")
            gx1o[d] = penc.tile([128, 3, NOPT], f32, name=f"gx1o{d}")
            _enc_projection(nc, ppj, xt_ctx, W[f"wihT_{d}"], W[f"bias3_{d}"],
                            gx1c[d], [(0, 400)])
            _enc_projection(nc, ppj, xt_opt, W[f"wihT_{d}"], W[f"bias3_{d}"],
                            gx1o[d], [(0, 500), (500, 500)])

        # ---- encoder scans ----
        ctx_o = {d: pm.tile([128, NCTX], f32, name=f"ctxo{d}") for d in ("f", "b")}
        opt_o = {d: pm.tile([128, NOPT], f32, name=f"opto{d}") for d in ("f", "b")}
        with ExitStack() as sctx:
            _gru_scan(nc, sctx, tc, LC, LCTX, gx1c["f"], gx1c["b"],
                      W["whhT_f"], W["whhT_b"], W["bhhn_f"], W["bhhn_b"],
                      outs=(ctx_o["f"], ctx_o["b"]), pfx="sc")
        with ExitStack() as sctx:
            _gru_scan(nc, sctx, tc, LO, LOPT, gx1o["f"], gx1o["b"],
                      W["whhT_f"], W["whhT_b"], W["bhhn_f"], W["bhhn_b"],
                      outs=(opt_o["f"], opt_o["b"]), pfx="so")
        if DEBUG:
            nc.sync.dma_start(dbg["d_ctxf"][:], ctx_o["f"][:])
            nc.sync.dma_start(dbg["d_ctxb"][:], ctx_o["b"][:])
            nc.sync.dma_start(dbg["d_optf"][:], opt_o["f"][:])
            nc.sync.dma_start(dbg["d_optb"][:], opt_o["b"][:])
        penc_ctx.close()

        # ---- f32r copies of encoder outputs ----
        ctxR = {d: pm.tile([128, NCTX], f32r, name=f"ctxR{d}") for d in ("f", "b")}
        optR = {d: pm.tile([128, NOPT], f32r, name=f"optR{d}") for d in ("f", "b")}
        for d in ("f", "b"):
            nc.gpsimd.dma_start(ctxR[d][:], ctx_o[d][:])
            nc.gpsimd.dma_start(optR[d][:], opt_o[d][:])

        # ---- ck / q projections (family P, f32r) ----
        ck = pm.tile([128, 2, NCTX], f32, name="ck")
        qq = pm.tile([128, 2, NOPT], f32, name="qq")
        for mc in range(2):
            ps = ppj.tile([128, 512], f32, tag="proj", name="ck_ps")
            for kc, d in enumerate(("f", "b")):
                nc.tensor.matmul(ps[:, 0:NCTX],
                                 W["wkT"][kc][:, mc * 128:(mc + 1) * 128],
                                 ctxR[d][:], start=(kc == 0), stop=(kc == 1))
            nc.vector.tensor_copy(ck[:, mc, :], ps[:, 0:NCTX])
            for s in range(2):
                ps2 = ppj.tile([128, 512], f32, tag="proj", name="q_ps")
                for kc, d in enumerate(("f", "b")):
                    nc.tensor.matmul(ps2[:, 0:500],
                                     W["wqT"][kc][:, mc * 128:(mc + 1) * 128],
                                     optR[d][:, s * 500:(s + 1) * 500],
                                     start=(kc == 0), stop=(kc == 1))
                nc.vector.tensor_copy(qq[:, mc, s * 500:(s + 1) * 500], ps2[:, 0:500])

        if DEBUG:
            nc.sync.dma_start(dbg["d_ck"][:], ck[:])
            nc.sync.dma_start(dbg["d_q"][:], qq[:])
            nc.sync.dma_start(dbg["d_we"][:], we[:])
        # ---- energy + attention per (b, k) pair ----
        # pair lane l = k*4 + b; opt cols (i, l) = i*20+l; ctx cols (c, b) = c*4+b
        acx = pm.tile([128, 2, LC, LOPT], f32, name="acx")   # attn_ctx, col (c, l)
        aop = pm.tile([128, 2, NFO], f32, name="aop")        # attn_opt, col (i, l)
        NB = 10  # i-block size
        pj_ctx.close()
        with ExitStack() as ectx:
            pe_s = ectx.enter_context(tc.tile_pool(name="pe_s", bufs=3))
            pe_m = ectx.enter_context(tc.tile_pool(name="pe_m", bufs=4))
            pp_sc = ectx.enter_context(tc.tile_pool(name="pp_sc", bufs=2, space="PSUM"))
            pp_at = ectx.enter_context(tc.tile_pool(name="pp_at", bufs=2, space="PSUM"))
            pp_tr = ectx.enter_context(tc.tile_pool(name="pp_tr", bufs=2, space="PSUM"))
            pdram = ectx.enter_context(tc.tile_pool(name="pdram", bufs=2, space="DRAM"))
            for l in range(LOPT):
                kk, bb = divmod(l, B4)
                Et = pe_m.tile([LO, LC], f32, tag="E", name="Et")
                dsc = pdram.tile([1, LO * LC], f32, tag="dsc", name="dsc")
                for ib in range(LO // NB):
                    ssc = pp_sc.tile([1, 2, 512], f32, tag="ssc", name="ssc")
                    for hc in range(2):
                        SA = pe_s.tile([128, NB, LC], f32, tag="SA", name="SA")
                        S = pe_s.tile([128, NB, LC], S_DT, tag="S", name="S")
                        q_view = bass.AP(
                            tensor=qq.tensor, offset=qq.offset
                            + (hc * NOPT + ib * NB * LOPT + l),
                            ap=[list(qq.ap[0]), [LOPT, NB], [0, LC]])
                        ck_view = bass.AP(
                            tensor=ck.tensor, offset=ck.offset + (hc * NCTX + bb),
                            ap=[list(ck.ap[0]), [0, NB], [B4, LC]])
                        nc.vector.tensor_add(SA[:], q_view, ck_view)
                        nc.scalar.activation(S[:], SA[:], AF.Tanh)
                        if DEBUG and l == 0 and ib == 0 and hc == 0:
                            nc.sync.dma_start(dbg["d_S0"][:], SA[:])
                        for j in range(2):
                            nc.tensor.matmul(
                                ssc[:, j, 0:500],
                                weRep[hc][:, 0:1],
                                S[:].rearrange("p a b -> p (a b)")[:, j * 500:(j + 1) * 500],
                                start=(hc == 0), stop=(hc == 1))
                    srow = pe_m.tile([1, NB * LC], f32, tag="srow", name="srow")
                    nc.vector.tensor_copy(
                        srow[:].rearrange("p (a b) -> p a b", b=500),
                        ssc[:, :, 0:500])
                    nc.sync.dma_start(
                        dsc[:, ib * NB * LC:(ib + 1) * NB * LC], srow[:])
                nc.sync.dma_start(
                    Et[:], dsc[:].rearrange("p (a b) -> (p a) b", b=LC))
                if DEBUG and l == 0:
                    nc.sync.dma_start(dbg["d_scores"][:], Et[:])
                nc.scalar.activation(Et[:], Et[:], AF.Exp)
                if DEBUG and l == 0:
                    nc.sync.dma_start(dbg["d_E"][:], Et[:])
                # sm_c = E / rowsum  (softmax over c, used for attn_opt)
                sc_sum = pe_m.tile([LO, 1], f32, tag="scs", name="sc_sum")
                nc.vector.tensor_reduce(sc_sum[:], Et[:], mybir.AxisListType.X, ALU.add)
                nc.vector.reciprocal(sc_sum[:], sc_sum[:])
                smc = pe_m.tile([LO, LC], f32, tag="smc", name="smc")
                nc.vector.tensor_scalar(smc[:], Et[:], sc_sum[:], None, ALU.mult)
                # transpose sm_c -> [100, 50] for the attn_opt matmul rhs
                smcT_ps = pp_tr.tile([LC, 128], f32, tag="tr", name="smcT_ps")[:, 0:LO]
                nc.tensor.transpose(smcT_ps[:], smc[:], ident[0:LO, 0:LO])
                smcT = pe_m.tile([LC, LO], f32r, tag="smcTs", name="smcT")
                nc.vector.tensor_copy(smcT[:], smcT_ps[:])
                # col sums S_i[c] replicated to all 128 partitions via a
                # ones [50,128] stationary (no partition broadcast needed)
                EtR = pe_m.tile([LO, LC], f32r, tag="EtR", name="EtR")
                nc.gpsimd.dma_start(EtR[:], Et[:])
                onesR = W.get("_onesR")
                if onesR is None:
                    ones32 = pw.tile([128, 128], f32, name="ones32")
                    nc.vector.memset(ones32[:], 1.0)
                    onesR = pw.tile([128, 128], f32r, name="onesR")
                    nc.gpsimd.dma_start(onesR[:], ones32[:])
                    W["_onesR"] = onesR
                si_ps = pp_at.tile([128, 512], f32, tag="at", name="si_ps")
                nc.tensor.matmul(si_ps[:, 0:LC], onesR[0:LO, :], EtR[:],
                                 start=True, stop=True)
                rsi_b = pe_m.tile([128, LC], f32, tag="rsib", name="rsi_b")
                nc.vector.reciprocal(rsi_b[:], si_ps[:, 0:LC])
                # attn_ctx = (opt_B.T @ E) * rsi_b ; attn_opt = ctx_B.T @ smcT
                for hc, d in enumerate(("f", "b")):
                    # opt_B / ctx_B slices via PE transpose of family-P outs
                    optB_ps = pp_tr.tile([LC, 128], f32, tag="tr", name="optB_ps")[0:LO, :]
                    in_view = bass.AP(tensor=opt_o[d].tensor,
                                      offset=opt_o[d].offset + l,
                                      ap=[list(opt_o[d].ap[0]), [LOPT, LO]])
                    nc.tensor.transpose(optB_ps[:], in_view, ident[:, :])
                    optB = pe_m.tile([LO, 128], f32r, tag="optB_s", name="optB")
                    nc.vector.tensor_copy(optB[:], optB_ps[:])
                    acx_ps = pp_at.tile([128, 512], f32, tag="at", name="acx_ps")
                    nc.tensor.matmul(acx_ps[:, 0:LC], optB[:], EtR[:],
                                     start=True, stop=True)
                    acx_view = bass.AP(
                        tensor=acx.tensor,
                        offset=acx.offset + (hc * LC * LOPT + l),
                        ap=[list(acx.ap[0]), [LOPT, LC]])
                    nc.vector.tensor_mul(acx_view, acx_ps[:, 0:LC], rsi_b[:])

                    ctxB_ps = pp_tr.tile([LC, 128], f32, tag="tr", name="ctxB_ps")
                    cin_view = bass.AP(tensor=ctx_o[d].tensor,
                                       offset=ctx_o[d].offset + bb,
                                       ap=[list(ctx_o[d].ap[0]), [B4, LC]])
                    nc.tensor.transpose(ctxB_ps[:], cin_view, ident[:, :])
                    ctxB = pe_m.tile([LC, 128], f32r, tag="ctxB_s", name="ctxB")
                    nc.vector.tensor_copy(ctxB[:], ctxB_ps[:])
                    aop_ps = pp_at.tile([128, 512], f32, tag="at", name="aop_ps")
                    nc.tensor.matmul(aop_ps[:, 0:LO], ctxB[:], smcT[:],
                                     start=True, stop=True)
                    aop_view = bass.AP(
                        tensor=aop.tensor,
                        offset=aop.offset + (hc * NFO + l),
                        ap=[list(aop.ap[0]), [LOPT, LO]])
                    nc.vector.tensor_copy(aop_view, aop_ps[:, 0:LO])
        if DEBUG:
            nc.sync.dma_start(dbg["d_acx"][:], acx[:].rearrange("p a b c -> p a (b c)"))
            nc.sync.dma_start(dbg["d_aop"][:], aop[:])

        # ---- attention-GRU: gx2 projections + scans, two shifts (fc, fo) ----
        hmx = {}
        for shift in ("fc", "fo"):
            T2 = LC if shift == "fc" else LO
            N2 = T2 * LOPT
            with ExitStack() as actx:
                pg2 = actx.enter_context(tc.tile_pool(name=f"pg2{shift}", bufs=1))
                pfs = actx.enter_context(tc.tile_pool(name=f"pfs{shift}", bufs=1))
                pp2 = actx.enter_context(tc.tile_pool(name=f"pp2{shift}", bufs=2,
                                                      space="PSUM"))
                gx2 = {d: pg2.tile([128, 3, N2], f32, name=f"gx2{shift}{d}")
                       for d in ("f", "b")}
                # feature chunk sources (family P, col = (t, l)); first build
                # per-slice f32r buffers, then accumulate the projection.
                if shift == "fc":
                    base = {d: ctx_o[d] for d in ("f", "b")}
                    baseR = {d: ctxR[d] for d in ("f", "b")}
                    att = acx[:].rearrange("p a b c -> p a (b c)")
                    bview = {d: bass.AP(
                        tensor=base[d].tensor, offset=base[d].offset,
                        ap=[list(base[d].ap[0]), [B4, LC], [0, KOPT], [1, B4]])
                        for d in ("f", "b")}
                else:
                    base = {d: opt_o[d] for d in ("f", "b")}
                    baseR = {d: optR[d] for d in ("f", "b")}
                    att = aop[:]
                    bview = {d: base[d][:].rearrange("p (t l) -> p t l", l=LOPT)
                             for d in ("f", "b")}
                nslices = N2 // 500
                for s in range(nslices):
                    # 8 feature chunks for this 500-col slice, f32r
                    fsl = []
                    for hc, d in enumerate(("f", "b")):
                        if shift == "fc":
                            fR = pfs.tile([128, 500], f32r, tag=f"fb{hc}", name=f"fb{hc}")
                            nc.vector.tensor_copy(
                                fR[:].rearrange("p (a b) -> p a b", b=LOPT)
                                .rearrange("p a (k c) -> p a k c", k=KOPT),
                                bview[d][:, s * 25:(s + 1) * 25, :, :])
                        else:
                            fR = bass.AP(tensor=baseR[d].tensor,
                                         offset=baseR[d].offset + s * 500,
                                         ap=[list(baseR[d].ap[0]), [1, 500]])
                        fsl.append(fR)
                    for hc in range(2):
                        aR = pfs.tile([128, 500], f32r, tag=f"fa{hc}", name=f"fa{hc}")
                        nc.gpsimd.dma_start(aR[:], att[:, hc, s * 500:(s + 1) * 500])
                        fsl.append(aR)
                    for hc, d in enumerate(("f", "b")):
                        pR = pfs.tile([128, 500], f32r, tag=f"fp{hc}", name=f"fp{hc}")
                        nc.vector.tensor_mul(
                            pR[:].rearrange("p (a b) -> p a b", b=LOPT)
                            if shift == "fc" else pR[:],
                            bview[d][:, s * 25:(s + 1) * 25, :, :]
                            if shift == "fc" else fsl[hc],
                            att[:, hc, s * 500:(s + 1) * 500].rearrange(
                                "p (a b) -> p a b", b=LOPT)
                            if shift == "fc" else att[:, hc, s * 500:(s + 1) * 500])
                        fsl.append(pR)
                    for hc, d in enumerate(("f", "b")):
                        dR = pfs.tile([128, 500], f32r, tag=f"fd{hc}", name=f"fd{hc}")
                        nc.vector.tensor_sub(
                            dR[:].rearrange("p (a b) -> p a b", b=LOPT)
                            if shift == "fc" else dR[:],
                            bview[d][:, s * 25:(s + 1) * 25, :, :]
                            if shift == "fc" else fsl[hc],
                            att[:, hc, s * 500:(s + 1) * 500].rearrange(
                                "p (a b) -> p a b", b=LOPT)
                            if shift == "fc" else att[:, hc, s * 500:(s + 1) * 500])
                        fsl.append(dR)
                    for d in ("f", "b"):
                        for g in range(3):
                            ps = pp2.tile([128, 512], f32, tag="p2", name="gx2_ps")
                            for fch in range(8):
                                nc.tensor.matmul(
                                    ps[:, 0:500],
                                    W[f"awihT_{d}"][fch][:, g * 128:(g + 1) * 128],
                                    fsl[fch] if isinstance(fsl[fch], bass.AP)
                                    else fsl[fch][:],
                                    start=(fch == 0), stop=(fch == 7))
                            nc.vector.tensor_scalar(
                                gx2[d][:, g, s * 500:(s + 1) * 500], ps[:, 0:500],
                                W[f"abias3_{d}"][:, g:g + 1], None, ALU.add)
                # running-max scan
                hmx[shift] = {d: pm.tile([128, LOPT], f32, name=f"hmx{shift}{d}")
                              for d in ("f", "b")}
                for d in ("f", "b"):
                    nc.vector.memset(hmx[shift][d][:], -1e30)
                with ExitStack() as sctx:
                    _gru_scan(nc, sctx, tc, T2, LOPT, gx2["f"], gx2["b"],
                              W["awhhT_f"], W["awhhT_b"],
                              W["abhhn_f"], W["abhhn_b"],
                              outs=None,
                              hmax=(hmx[shift]["f"], hmx[shift]["b"]),
                              pfx=f"sa{shift}")
        if DEBUG:
            nc.sync.dma_start(dbg["d_hcf"][:], hmx["fc"]["f"][:])
            nc.sync.dma_start(dbg["d_hcb"][:], hmx["fc"]["b"][:])
            nc.sync.dma_start(dbg["d_hof"][:], hmx["fo"]["f"][:])
            nc.sync.dma_start(dbg["d_hob"][:], hmx["fo"]["b"][:])

        # ---- logits + softmax over K ----
        with ExitStack() as lctx:
            plg = lctx.enter_context(tc.tile_pool(name="plg", bufs=1))
            pplg = lctx.enter_context(tc.tile_pool(name="pplg", bufs=1, space="PSUM"))
            featR = []
            for shift in ("fc", "fo"):
                for d in ("f", "b"):
                    fR = plg.tile([128, LOPT], f32r, name=f"featR{shift}{d}")
                    nc.gpsimd.dma_start(fR[:], hmx[shift][d][:])
                    featR.append(fR)
            lg_ps = pplg.tile([1, 512], f32, name="lg_ps")
            for i, fR in enumerate(featR):
                nc.tensor.matmul(lg_ps[:, 0:LOPT], W["wsimT"][i], fR[:],
                                 start=(i == 0), stop=(i == 3))
            lg_row = plg.tile([1, LOPT], f32, name="lg_row")
            nc.vector.tensor_copy(lg_row[:], lg_ps[:, 0:LOPT])
            if DEBUG:
                nc.sync.dma_start(dbg["d_logits"][:], lg_row[:])
            pldram = lctx.enter_context(tc.tile_pool(name="pldram", bufs=1,
                                                     space="DRAM"))
            dlg = pldram.tile([1, LOPT], f32, name="dlg")
            nc.sync.dma_start(dlg[:], lg_row[:])
            lg = plg.tile([B4, KOPT], f32, name="lg")
            nc.sync.dma_start(
                lg[:], bass.AP(tensor=dlg.tensor, offset=dlg.offset,
                               ap=[[1, B4], [B4, KOPT]]))
            mx = plg.tile([B4, 1], f32, name="mx")
            nc.vector.tensor_reduce(mx[:], lg[:], mybir.AxisListType.X, ALU.max,
                                    negate=True)
            ex = plg.tile([B4, KOPT], f32, name="ex")
            sm = plg.tile([B4, 1], f32, name="sm")
            nc.scalar.activation(ex[:], lg[:], AF.Exp, bias=mx[:], accum_out=sm[:])
            nc.vector.reciprocal(sm[:], sm[:])
            prob = plg.tile([B4, KOPT], f32, name="prob")
            nc.vector.tensor_scalar(prob[:], ex[:], sm[:], None, ALU.mult)
            nc.sync.dma_start(out_ap[:], prob[:])

    split_multi_waits(nc)
    return nc


def _prep_weights(inputs):
    """Host-side weight marshalling (layouts only, plus bias folding)."""
    g = {k: np.asarray(v, dtype=np.float32) for k, v in inputs.items()
         if k not in ("context", "options", "context_lens", "option_lens")}
    wm = {}
    for d, sfx in (("f", "_f"), ("b", "_b")):
        wm[f"wihT_{d}"] = np.ascontiguousarray(g["W_ih" + sfx].T)        # [300, 384]
        wm[f"whhT_{d}"] = np.ascontiguousarray(g["W_hh" + sfx].T)        # [128, 384]
        bih, bhh = g["b_ih" + sfx], g["b_hh" + sfx]
        b3 = np.stack([bih[0:128] + bhh[0:128],
                       bih[128:256] + bhh[128:256],
                       bih[256:384]], axis=1)                            # [128, 3]
        wm[f"bias3_{d}"] = np.ascontiguousarray(b3)
        wm[f"bhhn_{d}"] = np.ascontiguousarray(bhh[256:384][:, None])    # [128, 1]
        wm[f"awihT_{d}"] = np.ascontiguousarray(g["aW_ih" + sfx].T)      # [1024, 384]
        wm[f"awhhT_{d}"] = np.ascontiguousarray(g["aW_hh" + sfx].T)
        abih, abhh = g["ab_ih" + sfx], g["ab_hh" + sfx]
        ab3 = np.stack([abih[0:128] + abhh[0:128],
                        abih[128:256] + abhh[128:256],
                        abih[256:384]], axis=1)
        wm[f"abias3_{d}"] = np.ascontiguousarray(ab3)
        wm[f"abhhn_{d}"] = np.ascontiguousarray(abhh[256:384][:, None])
    wm["wkT"] = np.ascontiguousarray(g["Wk"].T)
    wm["wqT"] = np.ascontiguousarray(g["Wq"].T)
    wm["wemat"] = np.ascontiguousarray(g["We"])
    wm["vvec"] = np.ascontiguousarray(g["v"][:, None])
    wm["wsimT"] = np.ascontiguousarray(g["Wsim"][0][:, None])            # [512, 1]
    return wm


def kernel(**inputs):
    if "nc" not in _BUILT:
        _BUILT["nc"] = _build()
    nc = _BUILT["nc"]
    context = np.asarray(inputs["context"], dtype=np.float32)   # [32, 100, 300]
    options = np.asarray(inputs["options"], dtype=np.float32)   # [32, 5, 50, 300]
    wm = _prep_weights(inputs)
    B = context.shape[0]
    in_maps = []
    for c in range(NCORES):
        bs = slice(c * B4, (c + 1) * B4)
        ctx_sh = context[bs]                       # [4, 100, 300]
        opt_sh = options[bs]                       # [4, 5, 50, 300]
        m = dict(wm)
        # (e, (t, b)) and (e, (t, k*4+b))
        m["ctxT"] = np.ascontiguousarray(ctx_sh.transpose(2, 1, 0).reshape(E, NCTX))
        m["optT"] = np.ascontiguousarray(
            opt_sh.transpose(3, 2, 1, 0).reshape(E, NOPT))
        in_maps.append(m)
    res = run_bass_kernel_spmd(nc, in_maps, list(range(NCORES)))
    out = np.concatenate([res.results[c]["out"] for c in range(NCORES)], axis=0)
    if DEBUG:
        kernel.debug = [res.results[c] for c in range(NCORES)]
    return out.astype(np.float32)



# revision 40
# speedup vs baseline: 2980.8237x; 2980.8237x over previous
"""Trainium2 Bass kernel for nn_BiGruBNattMaxFocalNet.

Data-parallel over batch: B=32 -> 4 per core x 8 cores.
Per-core pipeline (all feature-on-partition "family P" layouts):
  1. encoder input projections (f32r matmuls)
  2. shared BiGRU scans over context (T=100, L=4) and options (T=50, L=20)
  3. ctx_key / query projections
  4. per-(b,k) Bahdanau energy: DVE outer-add + ACT tanh + PE w_e-reduce
  5. exp / row-col sums / normalized attention matmuls
  6. fc/fo features -> attention-GRU input projections
  7. attention BiGRU scans with running max
  8. logits + softmax over K
"""
import numpy as np
from contextlib import ExitStack

import concourse.bass as bass
import concourse.tile as tile
from concourse import mybir, masks
from concourse.bass_utils import run_bass_kernel_spmd
from concourse.vector_clock import ScopedClock

f32 = mybir.dt.float32
f32r = mybir.dt.float32r
bf16 = mybir.dt.bfloat16
AF = mybir.ActivationFunctionType
ALU = mybir.AluOpType

H, H2, H3, E, LC, LO, KOPT = 128, 256, 384, 300, 100, 50, 5
NCORES = 8
B4 = 4            # batch per core
LCTX = B4         # ctx scan lanes
LOPT = B4 * KOPT  # option scan lanes (=20)
NCTX = LC * LCTX      # 400 ctx (t,b) cols
NOPT = LO * LOPT      # 1000 opt (t,l) cols
NFC = LC * LOPT       # 2000 fc cols
NFO = LO * LOPT       # 1000 fo cols
WPACK_N = 4 * H3 + 4 * 3 + 4 * 1 + 2  # packed small fp32 weights

# dtype used for the tanh-energy tiles consumed by the w_e-reduce matmul
# (f32r keeps the reduce at 1 cycle/row with ~1e-4 rounding)
S_DT = f32r


class TC(tile.TileContext):
    """TileContext with walrus-compatible tail drain (<=1 wait per inst)."""

    def _drain_and_barrier(self, tick_clock, wait_clock):
        nc = self.nc
        probe = nc.sync.nop(nofuse=True)
        wait_clock.add_sem_waits(
            probe.ins, ScopedClock({None: tick_clock.global_clock})
        )
        si = probe.ins.sync_info
        waits = list(si.on_wait or [])
        si.on_wait = []
        assert self.sems is not None
        by_name = {h.name: h for h in self.sems.allocated().values()}
        for w in waits:
            nc.sync.wait_ge(by_name[w.ant_name], w.wait_value)
        nc.sync.drain()
        nc.all_engine_barrier()
        popped = nc._tile_sem_poison_stack.pop()
        assert popped is self._sem_poison
        nc.clear_and_free_semaphores(list(self.sems.allocated().values()))
        nc.all_engine_barrier()


def split_multi_waits(nc, max_waits=1):
    """This walrus build rejects >1 sync-wait per instruction; hoist extras
    onto same-engine NOPs placed immediately before the offender."""
    cnt = 0
    for fn in nc.m.functions:
        for bb in fn.blocks:
            insts = list(bb.instructions)
            out = []
            changed = False
            for inst in insts:
                si = inst.sync_info
                waits = list(si.on_wait) if si is not None and si.on_wait else []
                if len(waits) > max_waits:
                    changed = True
                    for w in waits[:-max_waits]:
                        cnt += 1
                        nop = mybir.InstNoOp(name=f"wait-split-{cnt}")
                        nop.engine = inst.engine
                        nop.sync_info = mybir.SyncInfo(on_wait=[w], on_update=[])
                        out.append(nop)
                    inst.sync_info = mybir.SyncInfo(
                        on_wait=waits[-max_waits:],
                        on_update=list(si.on_update or []),
                    )
                out.append(inst)
            if changed:
                bb.instructions = out
    return cnt


def _enc_projection(nc, ppj, xt_tiles, wihT, bias3, writes):
    """(x @ Wih.T + bias) gate chunks -> caller-provided output APs.

    writes: list of (lo, n, out_ap_fn) where out_ap_fn(g) is the [128, n]
    destination AP for gate g of input columns [lo, lo+n).
    """
    for (lo, n, out_ap_fn) in writes:
        for g in range(3):
            ps = ppj.tile([128, 512], f32, tag="proj", name="ps_proj")
            for kc, (xt, rows) in enumerate(xt_tiles):
                nc.tensor.matmul(
                    ps[:, 0:n],
                    wihT[kc][0:rows, g * 128:(g + 1) * 128],
                    xt[0:rows, lo:lo + n],
                    start=(kc == 0), stop=(kc == len(xt_tiles) - 1),
                )
            nc.vector.tensor_scalar(
                out_ap_fn(g), ps[:, 0:n], bias3[:, g:g + 1], None, ALU.add
            )


def _gru_scan2(nc, ctx, tc, groups, ident, pfx=""):
    """Merged multi-group bidirectional GRU scan (dirs fused per group).

    Each group dict:
      T, L    : steps / lanes per direction
      gx      : tile [128, 3, 2, T, L] f32. rows r,z include bih+bhh biases;
                row n is the bih_n-only input projection.
      bhhnL   : tile [128, 2, L] f32 (bhh_n broadcast along lanes, per dir)
      whhT    : {'f','b'} tiles [128, 384] f32
      outs    : None or {'f','b'} tiles [128, T*L] (h_t storage, col=(t,lane))
      hmax    : None or tile [128, 2, L] (running max, pre-initialized)
    Per step: PSUM P[128,3,2,L] <- Whh h (6 mm) + gx_rz (acc mm) + bhh_n
    (acc mm); sigmoid/tanh on ACT; elementwise split DVE/Pool.
    """
    hp = ctx.enter_context(tc.tile_pool(name=f"h{pfx}", bufs=3))
    vp = ctx.enter_context(tc.tile_pool(name=f"v{pfx}", bufs=4))
    gp = ctx.enter_context(tc.tile_pool(name=f"g{pfx}", bufs=3, space="PSUM"))

    for gi, G in enumerate(groups):
        G['h0'] = hp.tile([128, 2, G['L']], f32, tag=f"h0{gi}", name=f"h0{gi}")
        nc.vector.memset(G['h0'][:], 0.0)
        G['hprev'] = None  # scratch-group chaining

    def half1(gi, G, s):
        """MMs + sigmoid + tn + sn for step s of group gi."""
        T, L, gx = G['T'], G['L'], G['gx']
        if s == 0:
            hin = {d: G['h0'][:, di, :] for di, d in enumerate(("f", "b"))}
        elif G['outs'] is not None:
            hin = {"f": G['outs']["f"][:, (s - 1) * L:s * L],
                   "b": G['outs']["b"][:, (T - s) * L:(T - s + 1) * L]}
        else:
            hin = {d: G['hprev'][:, di, :] for di, d in enumerate(("f", "b"))}
        P = gp.tile([128, 3, 2, L], f32, tag=f"P{gi}", name=f"P{gi}")
        Pf = P[:].rearrange("p a b c -> p (a b c)")
        S = dict(hin=hin, P=P, Pf=Pf)
        G['st'] = S
        delta = L * (2 * T - 1 - 2 * s)
        # gx/bias accumulation first: no step deps, so it runs off-chain
        # during the previous step's tail
        rz_rhs = bass.AP(tensor=gx.tensor, offset=gx.offset + s * L,
                         ap=[list(gx.ap[0]), [2 * T * L, 2], [delta, 2], [1, L]])
        # single accumulation group per step (one psum bank): accs first
        # (no step deps -> run during the previous step's tail), gates last
        nc.tensor.matmul(Pf[:, 0:4 * L], ident[:], rz_rhs, start=True, stop=False)
        nc.tensor.matmul(Pf[:, 4 * L:6 * L], ident[:], G['bhhnL'][:],
                         start=False, stop=False)
        for di, d in enumerate(("f", "b")):
            for g3 in range(2):
                nc.tensor.matmul(
                    P[:, g3, di, :], G['whhT'][d][:, g3 * 128:(g3 + 1) * 128],
                    hin[d], start=False, stop=False)
        for di, d in enumerate(("f", "b")):
            nc.tensor.matmul(P[:, 2, di, :], G['whhT'][d][:, 256:384],
                             hin[d], start=False, stop=(di == 1))
        rz = vp.tile([128, 2, 2, L], f32, tag=f"rz{gi}", name=f"rz{gi}")
        S['rzf'] = rz[:].rearrange("p a b c -> p (a b c)")
        nc.scalar.activation(S['rzf'][:, 0:4 * L], Pf[:, 0:4 * L], AF.Sigmoid)
        tn = vp.tile([128, 2, L], f32, tag=f"tn{gi}", name=f"tn{gi}")
        nc.vector.tensor_mul(tn[:].rearrange("p a b -> p (a b)"),
                             Pf[:, 4 * L:6 * L], S['rzf'][:, 0:2 * L])
        sn = vp.tile([128, 2, L], f32, tag=f"sn{gi}", name=f"sn{gi}")
        gxn_rhs = bass.AP(
            tensor=gx.tensor, offset=gx.offset + 4 * T * L + s * L,
            ap=[list(gx.ap[0]), [delta, 2], [1, L]])
        nc.vector.tensor_add(sn[:], tn[:], gxn_rhs)
        S['sn'] = sn

    def half2(gi, G, s):
        """tanh + dd + ee + h2 (+hmax) for step s of group gi.

        Group 0's elementwise tail runs on DVE, group 1's on Pool, so the
        two chains' tails never contend for the same engine queue.
        """
        T, L = G['T'], G['L']
        S = G['st']
        eng = nc.vector if gi == 0 else nc.gpsimd
        n_t = vp.tile([128, 2, L], f32, tag=f"n{gi}", name=f"n{gi}")
        nc.scalar.activation(n_t[:], S['sn'][:], AF.Tanh)
        dd = vp.tile([128, 2, L], f32, tag=f"dd{gi}", name=f"dd{gi}")
        if G['outs'] is not None:
            eng.tensor_sub(dd[:, 0, :], S['hin']["f"], n_t[:, 0, :])
            eng.tensor_sub(dd[:, 1, :], S['hin']["b"], n_t[:, 1, :])
        else:
            eng.tensor_sub(
                dd[:], G['h0'][:] if s == 0 else G['hprev'][:], n_t[:])
        ee = vp.tile([128, 2, L], f32, tag=f"ee{gi}", name=f"ee{gi}")
        eng.tensor_mul(
            ee[:].rearrange("p a b -> p (a b)"), S['rzf'][:, 2 * L:4 * L],
            dd[:].rearrange("p a b -> p (a b)"))
        if G['outs'] is not None:
            eng.tensor_add(G['outs']["f"][:, s * L:(s + 1) * L],
                           n_t[:, 0, :], ee[:, 0, :])
            eng.tensor_add(G['outs']["b"][:, (T - 1 - s) * L:(T - s) * L],
                           n_t[:, 1, :], ee[:, 1, :])
        else:
            h2 = hp.tile([128, 2, L], f32, tag=f"h2{gi}", name=f"h2{gi}")
            eng.tensor_add(h2[:], n_t[:], ee[:])
            nc.vector.tensor_tensor(G['hmax'][:], G['hmax'][:], h2[:], ALU.max)
            G['hprev'] = h2

    Tmax = max(G['T'] for G in groups)
    for s in range(Tmax):
        act = [(gi, G) for gi, G in enumerate(groups) if s < G['T']]
        for gi, G in act:
            half1(gi, G, s)
        for gi, G in act:
            half2(gi, G, s)


DEBUG = False
_BUILT = {}


def _build(split=True):
    nc = bass.Bass("TRN2", target_bir_lowering=False, debug=False)
    dram = {}

    def din(name, shape):
        dram[name] = nc.dram_tensor(name, list(shape), f32, kind="ExternalInput").ap()
        return dram[name]

    # sharded activations (host pre-transposed, E padded to 384)
    din("ctxT", [384, NCTX])    # (e, (t, b))
    din("optT", [384, NOPT])    # (e, (t, k*4+b))
    # encoder weights
    for d in ("f", "b"):
        din(f"wihT_{d}", [384, H3])
        din(f"awihT_{d}", [8 * H, H3])
    din("wpack", [H, WPACK_N])
    din("wkT", [H2, H2])
    din("wqT", [H2, H2])
    din("wsimT", [4 * H, 1])
    out_ap = nc.dram_tensor("out", [B4, KOPT], f32, kind="ExternalOutput").ap()
    dbg = {}
    if DEBUG:
        for nm, shape in [
            ("d_ctxf", [H, NCTX]), ("d_ctxb", [H, NCTX]),
            ("d_optf", [H, NOPT]), ("d_optb", [H, NOPT]),
            ("d_scores", [LC, LO]), ("d_E", [LC, LO]),
            ("d_ck", [H, 2, NCTX]), ("d_q", [H, 2, NOPT]), ("d_we", [H, 2]),
            ("d_S0", [H, 10, LC]),
            ("d_acx", [H, 2, NFC]), ("d_aop", [H, 2, NFO]),
            ("d_hcf", [H, LOPT]), ("d_hcb", [H, LOPT]),
            ("d_hof", [H, LOPT]), ("d_hob", [H, LOPT]),
            ("d_logits", [1, LOPT]),
        ]:
            dbg[nm] = nc.dram_tensor(nm, shape, f32, kind="ExternalOutput").ap()

    with TC(nc) as tc, ExitStack() as ctx:
        pw = ctx.enter_context(tc.tile_pool(name="pw", bufs=1))
        pm = ctx.enter_context(tc.tile_pool(name="pm", bufs=1))
        pj_ctx = ExitStack()
        ppj = pj_ctx.enter_context(tc.tile_pool(name="ppj", bufs=2, space="PSUM"))

        # ---- load weights: few big SP-queue DMAs ----
        W = {}

        def chunked(dram_ap, nch, ncols):
            """[nch*128, ncols] dram -> AP shaped [128, nch, ncols]."""
            return bass.AP(tensor=dram_ap.tensor, offset=dram_ap.offset,
                           ap=[[ncols, 128], [128 * ncols, nch], [1, ncols]])

        wih = {}
        for d in ("f", "b"):
            wih[d] = pw.tile([128, 3, H3], f32r, name=f"wih{d}")
            nc.gpsimd.dma_start(wih[d][:], chunked(dram[f"wihT_{d}"], 3, H3))
            W[f"wihT_{d}"] = [wih[d][:, kc, :] for kc in range(3)]
        # small fp32 weights packed host-side into one [128, NPACK] tensor
        wpack = pw.tile([128, WPACK_N], f32, name="wpack")
        nc.sync.dma_start(wpack[:], dram["wpack"][:])
        off = 0
        for nm in ("whhT_f", "whhT_b", "awhhT_f", "awhhT_b"):
            W[nm] = wpack[:, off:off + H3]
            off += H3
        for nm in ("bias3_f", "bias3_b", "abias3_f", "abias3_b"):
            W[nm] = wpack[:, off:off + 3]
            off += 3
        for nm in ("bhhn_f", "bhhn_b", "abhhn_f", "abhhn_b"):
            W[nm] = wpack[:, off:off + 1]
            off += 1
        we = wpack[:, off:off + 2]
        off += 2
        ident = pw.tile([128, 128], f32, name="ident")
        masks.make_identity(nc, ident[:])

        # ---- load activations (f32r, E padded to 384 host-side) ----
        penc_ctx = ExitStack()
        penc = penc_ctx.enter_context(tc.tile_pool(name="penc", bufs=1))
        ctxT = penc.tile([128, 3, NCTX], f32r, name="ctxT")
        optT = penc.tile([128, 3, NOPT], f32r, name="optT")
        nc.gpsimd.dma_start(ctxT[:], chunked(dram["ctxT"], 3, NCTX))
        nc.gpsimd.dma_start(optT[:], chunked(dram["optT"], 3, NOPT))
        xt_ctx = [(ctxT[:, kc, :], 128) for kc in range(3)]
        xt_opt = [(optT[:, kc, :], 128) for kc in range(3)]
        for d in ("f", "b"):
            aw = pw.tile([128, 8, H3], f32r, name=f"awih{d}")
            nc.gpsimd.dma_start(aw[:], chunked(dram[f"awihT_{d}"], 8, H3))
            W[f"awihT_{d}"] = [aw[:, kc, :] for kc in range(8)]
        for nm in ("wkT", "wqT"):
            t = pw.tile([128, 2, H2], f32r, name=nm)
            nc.gpsimd.dma_start(t[:], chunked(dram[nm], 2, H2))
            W[nm] = [t[:, kc, :] for kc in range(2)]
        wsim = pw.tile([128, 4], f32r, name="wsim")
        nc.gpsimd.dma_start(wsim[:], chunked(dram["wsimT"], 4, 1))
        W["wsimT"] = [wsim[:, kc:kc + 1] for kc in range(4)]

        # ---- encoder gx (merged-dir layouts) ----
        gxC = penc.tile([128, 3, 2, LC, LCTX], f32, name="gxC")
        gxO = penc.tile([128, 3, 2, LO, LOPT], f32, name="gxO")
        for di, d in enumerate(("f", "b")):
            _enc_projection(
                nc, ppj, xt_ctx, W[f"wihT_{d}"], W[f"bias3_{d}"],
                [(0, 400, lambda g, di=di: gxC[:, g, di, :, :])])
            _enc_projection(
                nc, ppj, xt_opt, W[f"wihT_{d}"], W[f"bias3_{d}"],
                [(0, 500, lambda g, di=di: gxO[:, g, di, 0:25, :]),
                 (500, 500, lambda g, di=di: gxO[:, g, di, 25:50, :])])
        # bhh_n lane-broadcast tiles
        bhhnL_c = pw.tile([128, 2, LCTX], f32, name="bhhnLc")
        bhhnL_o = pw.tile([128, 2, LOPT], f32, name="bhhnLo")
        for di, d in enumerate(("f", "b")):
            bh = W[f"bhhn_{d}"]
            nc.vector.tensor_copy(
                bhhnL_c[:, di, :],
                bass.AP(tensor=bh.tensor, offset=bh.offset,
                        ap=[list(bh.ap[0]), [0, LCTX]]))
            nc.vector.tensor_copy(
                bhhnL_o[:, di, :],
                bass.AP(tensor=bh.tensor, offset=bh.offset,
                        ap=[list(bh.ap[0]), [0, LOPT]]))

        # ---- merged encoder scan (ctx group + opt group) ----
        ctx_o = {d: pm.tile([128, NCTX], f32, name=f"ctxo{d}") for d in ("f", "b")}
        opt_o = {d: pm.tile([128, NOPT], f32, name=f"opto{d}") for d in ("f", "b")}
        whh = {'f': W["whhT_f"], 'b': W["whhT_b"]}
        with ExitStack() as sctx:
            _gru_scan2(nc, sctx, tc, [
                dict(T=LC, L=LCTX, gx=gxC, bhhnL=bhhnL_c, whhT=whh,
                     outs=ctx_o, hmax=None),
                dict(T=LO, L=LOPT, gx=gxO, bhhnL=bhhnL_o, whhT=whh,
                     outs=opt_o, hmax=None),
            ], ident, pfx="e")
        if DEBUG:
            nc.sync.dma_start(dbg["d_ctxf"][:], ctx_o["f"][:])
            nc.sync.dma_start(dbg["d_ctxb"][:], ctx_o["b"][:])
            nc.sync.dma_start(dbg["d_optf"][:], opt_o["f"][:])
            nc.sync.dma_start(dbg["d_optb"][:], opt_o["b"][:])
        penc_ctx.close()

        # ---- f32r copies of encoder outputs (DMA performs f32r rounding) ----
        ctxRt = {d: pm.tile([128, NCTX], f32r, name=f"ctxR{d}") for d in ("f", "b")}
        optRt = {d: pm.tile([128, NOPT], f32r, name=f"optR{d}") for d in ("f", "b")}
        for d in ("f", "b"):
            nc.gpsimd.dma_start(ctxRt[d][:], ctx_o[d][:])
            nc.gpsimd.dma_start(optRt[d][:], opt_o[d][:])
        ctxR = {d: ctxRt[d][:] for d in ("f", "b")}
        optR = {d: optRt[d][:] for d in ("f", "b")}

        # ---- ck / q projections (family P, f32r) ----
        ck = pm.tile([128, 2, NCTX], f32, name="ck")
        qq = pm.tile([128, 2, NOPT], f32, name="qq")
        for mc in range(2):
            ps = ppj.tile([128, 512], f32, tag="proj", name="ck_ps")
            for kc, d in enumerate(("f", "b")):
                nc.tensor.matmul(ps[:, 0:NCTX],
                                 W["wkT"][kc][:, mc * 128:(mc + 1) * 128],
                                 ctxR[d], start=(kc == 0), stop=(kc == 1))
            nc.vector.tensor_copy(ck[:, mc, :], ps[:, 0:NCTX])
            for s in range(2):
                ps2 = ppj.tile([128, 512], f32, tag="proj", name="q_ps")
                for kc, d in enumerate(("f", "b")):
                    nc.tensor.matmul(ps2[:, 0:500],
                                     W["wqT"][kc][:, mc * 128:(mc + 1) * 128],
                                     optR[d][:, s * 500:(s + 1) * 500],
                                     start=(kc == 0), stop=(kc == 1))
                nc.vector.tensor_copy(qq[:, mc, s * 500:(s + 1) * 500], ps2[:, 0:500])

        if DEBUG:
            nc.sync.dma_start(dbg["d_ck"][:], ck[:])
            nc.sync.dma_start(dbg["d_q"][:], qq[:])
            nc.sync.dma_start(dbg["d_we"][:], we)
        # ---- energy + attention per (b, k) pair (bf16 core, v2) ----
        # pair lane l = k*4 + b; opt cols (i, l) = i*20+l; ctx cols (c, b) = c*4+b
        acx = pm.tile([128, 2, LC, LOPT], f32, name="acx")   # attn_ctx, col (c, l)
        aop = pm.tile([128, 2, NFO], f32, name="aop")        # attn_opt, col (i, l)
        pj_ctx.close()
        with ExitStack() as ectx:
            pe_pre = ectx.enter_context(tc.tile_pool(name="pe_pre", bufs=1))
            pe_s = ectx.enter_context(tc.tile_pool(name="pe_s", bufs=2))
            pe_ck = ectx.enter_context(tc.tile_pool(name="pe_ck", bufs=2))
            pe_m = ectx.enter_context(tc.tile_pool(name="pe_m", bufs=3))
            pp_sc = ectx.enter_context(tc.tile_pool(name="pp_sc", bufs=2, space="PSUM"))
            pp_at = ectx.enter_context(tc.tile_pool(name="pp_at", bufs=2, space="PSUM"))
            pp_tr = ectx.enter_context(tc.tile_pool(name="pp_tr", bufs=1, space="PSUM"))
            # bf16 staging of q, ck, we, ones
            q16 = pe_pre.tile([128, 2, NOPT], bf16, name="q16")
            nc.vector.tensor_copy(q16[:], qq[:])
            ck16 = pe_pre.tile([128, 2, NCTX], bf16, name="ck16")
            nc.vector.tensor_copy(ck16[:], ck[:])
            we16 = pe_pre.tile([128, 2], bf16, name="we16")
            nc.vector.tensor_copy(we16[:], we)
            ones16 = pe_pre.tile([128, 128], bf16, name="ones16")
            nc.vector.memset(ones16[:], 1.0)
            ident16 = pe_pre.tile([128, 128], bf16, name="ident16")
            nc.vector.tensor_copy(ident16[:], ident[:])
            # hoisted ctx transposes: ctxB16[(bb, d)] [LC, 128] bf16
            ctxB16 = {}
            for bb in range(B4):
                for d in ("f", "b"):
                    ps = pp_tr.tile([LC, 128], f32, tag="tr3", name="ctxB_ps")
                    cin = bass.AP(tensor=ctx_o[d].tensor,
                                  offset=ctx_o[d].offset + bb,
                                  ap=[list(ctx_o[d].ap[0]), [B4, LC]])
                    nc.tensor.transpose(ps[:], cin, ident[:, :])
                    t = pe_pre.tile([LC, 128], bf16, name=f"ctxB{bb}{d}")
                    nc.vector.tensor_copy(t[:], ps[:])
                    ctxB16[(bb, d)] = t
            for bb in range(B4):
              # ck broadcast over i for this batch lane: [128, 2, LC, LO] bf16
              ckrep = pe_ck.tile([128, 2, LC, LO], bf16, tag="ckrep",
                                 name="ckrep")
              csrc = bass.AP(tensor=ck16.tensor, offset=ck16.offset + bb,
                             ap=[list(ck16.ap[0]), [NCTX, 2], [B4, LC], [0, LO]])
              nc.vector.tensor_copy(ckrep[:], csrc)
              for kk in range(KOPT):
                l = kk * B4 + bb
                # ql16: [128, 2, LO] contiguous gather of this lane's queries
                ql16 = pe_m.tile([128, 2, LO], bf16, tag="ql", name="ql16")
                qsrc = bass.AP(tensor=q16.tensor, offset=q16.offset + l,
                               ap=[list(q16.ap[0]), [NOPT, 2], [LOPT, LO]])
                nc.vector.tensor_copy(ql16[:], qsrc)
                # S16[:, hc, c, i] = tanh(ck[c] + q[i]), bf16, in-place tanh
                S16 = pe_s.tile([128, 2, LC, LO], bf16, tag="S16", name="S16")
                for hc in range(2):
                    qv = bass.AP(tensor=ql16.tensor, offset=ql16.offset + hc * LO,
                                 ap=[list(ql16.ap[0]), [0, LC], [1, LO]])
                    nc.vector.tensor_add(S16[:, hc, :, :], ckrep[:, hc, :, :], qv)
                    nc.scalar.activation(S16[:, hc, :, :], S16[:, hc, :, :],
                                         AF.Tanh)
                # weighted feature reduce -> score row chunks -> [c, i] tile
                S16f = S16[:].rearrange("p a b c -> p (a b c)")
                EtTraw = pe_m.tile([LC, LO], f32, tag="EtT", name="EtTraw")
                for cc in range(5):
                    ssc = pp_sc.tile([1, 2, 512], f32, tag="ssc", name="ssc")
                    for j in range(2):
                        for hc in range(2):
                            base = hc * 5000 + cc * 1000 + j * 500
                            nc.tensor.matmul(
                                ssc[:, j, 0:500], we16[:, hc:hc + 1],
                                S16f[:, base:base + 500],
                                start=(hc == 0), stop=(hc == 1))
                    srow = pe_m.tile([1, 1000], f32, tag="srow", name="srow")
                    dst = bass.AP(tensor=srow.tensor, offset=srow.offset,
                                  ap=[list(srow.ap[0]), [500, 2], [1, 500]])
                    if cc < 3:
                        nc.vector.tensor_copy(dst, ssc[:, :, 0:500])
                    else:
                        nc.scalar.copy(dst, ssc[:, :, 0:500])
                    # diagonal SBUF->SBUF dma: row block -> EtTraw rows
                    nc.sync.dma_start(
                        EtTraw[cc * 20:(cc + 1) * 20, :],
                        bass.AP(tensor=srow.tensor, offset=srow.offset,
                                ap=[list(srow.ap[0]), [LO, 20], [1, LO]]))
                if DEBUG and l == 0:
                    nc.sync.dma_start(dbg["d_scores"][:], EtTraw[:])
                EtT16 = pe_m.tile([LC, LO], bf16, tag="EtT16", name="EtT16")
                nc.scalar.activation(EtT16[:], EtTraw[:], AF.Exp)
                # rowsums over c (replicated via ones-matmul) -> 1/rowsum
                rs_ps = pp_at.tile([128, 512], f32, tag="at", name="rs_ps")
                nc.tensor.matmul(rs_ps[:, 0:LO], ones16[0:LC, :], EtT16[:],
                                 start=True, stop=True)
                rsr = pe_m.tile([128, LO], f32, tag="rsr", name="rsr")
                nc.vector.reciprocal(rsr[:], rs_ps[:, 0:LO])
                smcT16 = pe_m.tile([LC, LO], bf16, tag="smcT", name="smcT16")
                nc.vector.tensor_mul(smcT16[:], EtT16[:], rsr[0:LC, :])
                # EtR [i, c] via PE transpose of exp'd scores
                tr_ps = pp_tr.tile([LO, 128], bf16, tag="tr2", name="EtR_ps")
                nc.tensor.transpose(tr_ps[:, 0:LC], EtT16[:], ident16[0:LC, 0:LC])
                EtR16 = pe_m.tile([LO, LC], bf16, tag="EtR", name="EtR16")
                nc.vector.tensor_copy(EtR16[:], tr_ps[:, 0:LC])
                # colsums over i (replicated) -> 1/colsum
                si_ps = pp_at.tile([128, 512], f32, tag="at", name="si_ps")
                nc.tensor.matmul(si_ps[:, 0:LC], ones16[0:LO, :], EtR16[:],
                                 start=True, stop=True)
                rsi = pe_m.tile([128, LC], f32, tag="rsi", name="rsi")
                nc.vector.reciprocal(rsi[:], si_ps[:, 0:LC])
                for hc, d in enumerate(("f", "b")):
                    oB_ps = pp_tr.tile([LO, 128], f32, tag="tr3", name="optB_ps")
                    oin = bass.AP(tensor=opt_o[d].tensor,
                                  offset=opt_o[d].offset + l,
                                  ap=[list(opt_o[d].ap[0]), [LOPT, LO]])
                    nc.tensor.transpose(oB_ps[:], oin, ident[:, :])
                    optB16 = pe_m.tile([LO, 128], bf16, tag=f"optB{hc}",
                                       name="optB16")
                    nc.vector.tensor_copy(optB16[:], oB_ps[:])
                    acx_ps = pp_at.tile([128, 512], f32, tag="at", name="acx_ps")
                    nc.tensor.matmul(acx_ps[:, 0:LC], optB16[:], EtR16[:],
                                     start=True, stop=True)
                    acx_view = bass.AP(
                        tensor=acx.tensor,
                        offset=acx.offset + (hc * LC * LOPT + l),
                        ap=[list(acx.ap[0]), [LOPT, LC]])
                    nc.vector.tensor_mul(acx_view, acx_ps[:, 0:LC], rsi[:])
                    aop_ps = pp_at.tile([128, 512], f32, tag="at", name="aop_ps")
                    nc.tensor.matmul(aop_ps[:, 0:LO], ctxB16[(bb, d)][:],
                                     smcT16[:], start=True, stop=True)
                    aop_view = bass.AP(
                        tensor=aop.tensor,
                        offset=aop.offset + (hc * NFO + l),
                        ap=[list(aop.ap[0]), [LOPT, LO]])
                    nc.vector.tensor_copy(aop_view, aop_ps[:, 0:LO])
        if DEBUG:
            nc.sync.dma_start(dbg["d_acx"][:], acx[:].rearrange("p a b c -> p a (b c)"))
            nc.sync.dma_start(dbg["d_aop"][:], aop[:])

        # ---- attention-GRU: gx2 projections (both shifts), one merged scan ----
        hmx = {}
        pg2_ctx = ExitStack()
        pg2 = pg2_ctx.enter_context(tc.tile_pool(name="pg2", bufs=1))
        gx2m = {"fc": pg2.tile([128, 3, 2, LC, LOPT], f32, name="gx2fc"),
                "fo": pg2.tile([128, 3, 2, LO, LOPT], f32, name="gx2fo")}
        for shift in ("fc", "fo"):
            T2 = LC if shift == "fc" else LO
            N2 = T2 * LOPT
            with ExitStack() as actx:
                pfs = actx.enter_context(tc.tile_pool(name=f"pfs{shift}", bufs=1))
                pp2 = actx.enter_context(tc.tile_pool(name=f"pp2{shift}", bufs=2,
                                                      space="PSUM"))
                gx2 = gx2m[shift]
                # feature chunk sources (family P, col = (t, l)); first build
                # per-slice f32r buffers, then accumulate the projection.
                if shift == "fc":
                    base = {d: ctx_o[d] for d in ("f", "b")}
                    baseR = {d: ctxR[d] for d in ("f", "b")}
                    att = acx[:].rearrange("p a b c -> p a (b c)")
                    bview = {d: bass.AP(
                        tensor=base[d].tensor, offset=base[d].offset,
                        ap=[list(base[d].ap[0]), [B4, LC], [0, KOPT], [1, B4]])
                        for d in ("f", "b")}
                else:
                    base = {d: opt_o[d] for d in ("f", "b")}
                    baseR = {d: optR[d] for d in ("f", "b")}
                    att = aop[:]
                    bview = {d: base[d][:].rearrange("p (t l) -> p t l", l=LOPT)
                             for d in ("f", "b")}
                nslices = N2 // 500
                for s in range(nslices):
                    # 8 feature chunks for this 500-col slice, f32r
                    fsl = []
                    for hc, d in enumerate(("f", "b")):
                        if shift == "fc":
                            fR = pfs.tile([128, 500], f32r, tag=f"fb{hc}", name=f"fb{hc}")
                            nc.vector.tensor_copy(
                                fR[:].rearrange("p (a b) -> p a b", b=LOPT)
                                .rearrange("p a (k c) -> p a k c", k=KOPT),
                                bview[d][:, s * 25:(s + 1) * 25, :, :])
                        else:
                            fR = baseR[d][:, s * 500:(s + 1) * 500]
                        fsl.append(fR)
                    for hc in range(2):
                        aR = pfs.tile([128, 500], f32r, tag=f"fa{hc}", name=f"fa{hc}")
                        nc.gpsimd.dma_start(aR[:], att[:, hc, s * 500:(s + 1) * 500])
                        fsl.append(aR)
                    for hc, d in enumerate(("f", "b")):
                        pR = pfs.tile([128, 500], f32r, tag=f"fp{hc}", name=f"fp{hc}")
                        nc.vector.tensor_mul(
                            pR[:].rearrange("p (a b) -> p a b", b=LOPT)
                            if shift == "fc" else pR[:],
                            bview[d][:, s * 25:(s + 1) * 25, :, :]
                            if shift == "fc" else fsl[hc],
                            att[:, hc, s * 500:(s + 1) * 500].rearrange(
                                "p (a b) -> p a b", b=LOPT)
                            if shift == "fc" else att[:, hc, s * 500:(s + 1) * 500])
                        fsl.append(pR)
                    for hc, d in enumerate(("f", "b")):
                        dR = pfs.tile([128, 500], f32r, tag=f"fd{hc}", name=f"fd{hc}")
                        nc.vector.tensor_sub(
                            dR[:].rearrange("p (a b) -> p a b", b=LOPT)
                            if shift == "fc" else dR[:],
                            bview[d][:, s * 25:(s + 1) * 25, :, :]
                            if shift == "fc" else fsl[hc],
                            att[:, hc, s * 500:(s + 1) * 500].rearrange(
                                "p (a b) -> p a b", b=LOPT)
                            if shift == "fc" else att[:, hc, s * 500:(s + 1) * 500])
                        fsl.append(dR)
                    for di, d in enumerate(("f", "b")):
                        for g in range(3):
                            ps = pp2.tile([128, 512], f32, tag="p2", name="gx2_ps")
                            for fch in range(8):
                                nc.tensor.matmul(
                                    ps[:, 0:500],
                                    W[f"awihT_{d}"][fch][:, g * 128:(g + 1) * 128],
                                    fsl[fch] if isinstance(fsl[fch], bass.AP)
                                    else fsl[fch][:],
                                    start=(fch == 0), stop=(fch == 7))
                            nc.vector.tensor_scalar(
                                gx2[:, g, di, s * 25:(s + 1) * 25, :], ps[:, 0:500],
                                W[f"abias3_{d}"][:, g:g + 1], None, ALU.add)
        # merged attention scan (fc group + fo group) with running max
        bhhnL_a = pw.tile([128, 2, LOPT], f32, name="bhhnLa")
        for di, d in enumerate(("f", "b")):
            bh = W[f"abhhn_{d}"]
            nc.vector.tensor_copy(
                bhhnL_a[:, di, :],
                bass.AP(tensor=bh.tensor, offset=bh.offset,
                        ap=[list(bh.ap[0]), [0, LOPT]]))
        awhh = {'f': W["awhhT_f"], 'b': W["awhhT_b"]}
        for shift in ("fc", "fo"):
            hmx[shift] = pm.tile([128, 2, LOPT], f32, name=f"hmx{shift}")
            nc.vector.memset(hmx[shift][:], -1e30)
        with ExitStack() as sctx:
            _gru_scan2(nc, sctx, tc, [
                dict(T=LC, L=LOPT, gx=gx2m["fc"], bhhnL=bhhnL_a, whhT=awhh,
                     outs=None, hmax=hmx["fc"]),
                dict(T=LO, L=LOPT, gx=gx2m["fo"], bhhnL=bhhnL_a, whhT=awhh,
                     outs=None, hmax=hmx["fo"]),
            ], ident, pfx="a")
        pg2_ctx.close()
        if DEBUG:
            nc.sync.dma_start(dbg["d_hcf"][:], hmx["fc"][:, 0, :])
            nc.sync.dma_start(dbg["d_hcb"][:], hmx["fc"][:, 1, :])
            nc.sync.dma_start(dbg["d_hof"][:], hmx["fo"][:, 0, :])
            nc.sync.dma_start(dbg["d_hob"][:], hmx["fo"][:, 1, :])

        # ---- logits + softmax over K ----
        with ExitStack() as lctx:
            plg = lctx.enter_context(tc.tile_pool(name="plg", bufs=1))
            pplg = lctx.enter_context(tc.tile_pool(name="pplg", bufs=1, space="PSUM"))
            featR = []
            for shift in ("fc", "fo"):
                for di in range(2):
                    fR = plg.tile([128, LOPT], f32r, name=f"featR{shift}{di}")
                    nc.gpsimd.dma_start(fR[:], hmx[shift][:, di, :])
                    featR.append(fR)
            lg_ps = pplg.tile([1, 512], f32, name="lg_ps")
            for i, fR in enumerate(featR):
                nc.tensor.matmul(lg_ps[:, 0:LOPT], W["wsimT"][i], fR[:],
                                 start=(i == 0), stop=(i == 3))
            lg_row = plg.tile([1, LOPT], f32, name="lg_row")
            nc.vector.tensor_copy(lg_row[:], lg_ps[:, 0:LOPT])
            if DEBUG:
                nc.sync.dma_start(dbg["d_logits"][:], lg_row[:])
            pldram = lctx.enter_context(tc.tile_pool(name="pldram", bufs=1,
                                                     space="DRAM"))
            dlg = pldram.tile([1, LOPT], f32, name="dlg")
            nc.sync.dma_start(dlg[:], lg_row[:])
            lg = plg.tile([B4, KOPT], f32, name="lg")
            nc.sync.dma_start(
                lg[:], bass.AP(tensor=dlg.tensor, offset=dlg.offset,
                               ap=[[1, B4], [B4, KOPT]]))
            mx = plg.tile([B4, 1], f32, name="mx")
            nc.vector.tensor_reduce(mx[:], lg[:], mybir.AxisListType.X, ALU.max,
                                    negate=True)
            ex = plg.tile([B4, KOPT], f32, name="ex")
            sm = plg.tile([B4, 1], f32, name="sm")
            nc.scalar.activation(ex[:], lg[:], AF.Exp, bias=mx[:], accum_out=sm[:])
            nc.vector.reciprocal(sm[:], sm[:])
            prob = plg.tile([B4, KOPT], f32, name="prob")
            nc.vector.tensor_scalar(prob[:], ex[:], sm[:], None, ALU.mult)
            nc.sync.dma_start(out_ap[:], prob[:])

    if split:
        split_multi_waits(nc)
    return nc


def _prep_weights(inputs):
    """Host-side weight marshalling (layouts, bias folding, wpack assembly)."""
    g = {k: np.asarray(v, dtype=np.float32) for k, v in inputs.items()
         if k not in ("context", "options", "context_lens", "option_lens")}
    wm = {}
    pack = []
    b3s, bns = [], []
    for d, sfx in (("f", "_f"), ("b", "_b")):
        wT = np.zeros((384, H3), np.float32)
        wT[0:E] = g["W_ih" + sfx].T
        wm[f"wihT_{d}"] = wT
        bih, bhh = g["b_ih" + sfx], g["b_hh" + sfx]
        b3s.append(np.stack([bih[0:128] + bhh[0:128],
                             bih[128:256] + bhh[128:256],
                             bih[256:384]], axis=1))                     # [128, 3]
        bns.append(bhh[256:384][:, None])                                # [128, 1]
        wm[f"awihT_{d}"] = np.ascontiguousarray(g["aW_ih" + sfx].T)      # [1024, 384]
        abih, abhh = g["ab_ih" + sfx], g["ab_hh" + sfx]
        b3s.append(np.stack([abih[0:128] + abhh[0:128],
                             abih[128:256] + abhh[128:256],
                             abih[256:384]], axis=1))
        bns.append(abhh[256:384][:, None])
    # wpack order: whhT_f, whhT_b, awhhT_f, awhhT_b, bias3 x4, bhhn x4, we
    pack = [g["W_hh_f"].T, g["W_hh_b"].T, g["aW_hh_f"].T, g["aW_hh_b"].T,
            b3s[0], b3s[2], b3s[1], b3s[3],
            bns[0], bns[2], bns[1], bns[3]]
    w_e = g["We"].T @ g["v"]                                             # [256]
    pack.append(np.stack([w_e[0:128], w_e[128:256]], axis=1))            # [128, 2]
    wm["wpack"] = np.ascontiguousarray(np.concatenate(pack, axis=1))
    assert wm["wpack"].shape == (H, WPACK_N), wm["wpack"].shape
    wm["wkT"] = np.ascontiguousarray(g["Wk"].T)
    wm["wqT"] = np.ascontiguousarray(g["Wq"].T)
    wm["wsimT"] = np.ascontiguousarray(g["Wsim"][0][:, None])            # [512, 1]
    return wm


def kernel(**inputs):
    if "nc" not in _BUILT:
        _BUILT["nc"] = _build()
    nc = _BUILT["nc"]
    context = np.asarray(inputs["context"], dtype=np.float32)   # [32, 100, 300]
    options = np.asarray(inputs["options"], dtype=np.float32)   # [32, 5, 50, 300]
    wm = _prep_weights(inputs)
    B = context.shape[0]
    in_maps = []
    for c in range(NCORES):
        bs = slice(c * B4, (c + 1) * B4)
        ctx_sh = context[bs]                       # [4, 100, 300]
        opt_sh = options[bs]                       # [4, 5, 50, 300]
        m = dict(wm)
        # (e, (t, b)) and (e, (t, k*4+b)), e zero-padded to 384
        ct = np.zeros((384, NCTX), np.float32)
        ct[0:E] = ctx_sh.transpose(2, 1, 0).reshape(E, NCTX)
        ot = np.zeros((384, NOPT), np.float32)
        ot[0:E] = opt_sh.transpose(3, 2, 1, 0).reshape(E, NOPT)
        m["ctxT"] = ct
        m["optT"] = ot
        in_maps.append(m)
    res = run_bass_kernel_spmd(nc, in_maps, list(range(NCORES)))
    out = np.concatenate([res.results[c]["out"] for c in range(NCORES)], axis=0)
    if DEBUG:
        kernel.debug = [res.results[c] for c in range(NCORES)]
    return out.astype(np.float32)

